# revision 46
# baseline (speedup 1.0000x reference)
"""Causal multi-head self-attention (RoPE) Trainium2 Bass kernel (v12).

v12: norm_a (the av-bank PSUM evacuation) is emitted directly at i-block end
instead of chain-slotted into the next block's first step, so the next
block's first AV matmul -- which reuses those av banks -- finds its copies
already drained through the DVE FIFO (was ~1us x 16 block boundaries).
Warmup extended to ~3.9us to bridge the DMA-paced prologue without a HAM
re-throttle; the last batch's y stores ride the by-then-empty SP ring.

Problem: x:(4,2048,1024), Wq/Wk/Wv:(1024,1024), Wo:(1024,1024), bo:(1024,)
  q,k,v = split_heads(x@W*), rope(q), rope(k), causal softmax(q k^T/8) v, @Wo+bo

Sharding: head-parallel across 8 cores. Core c owns heads {2c, 2c+1} for all
4 batches: it computes q/k/v projections against the 128-column weight slice,
attention for its heads, and a partial output projection against the matching
128-row slice of Wo. Host sums the 8 partial (8192,1024) fp16 outputs and
adds bo.

Changes over the 417us/384us baseline (trace-driven; final ~359us):
  - DMA split by traffic class over separate rings (the baseline pushed all
    43k packets through qSP-HWDGE): y stores ride the gpsimd SWDGE queue
    (idle engine, so its FIFO parking on ysb-ready waits is harmless), xT
    loads trigger from the near-idle SP sequencer, weights prologue on
    qACT-HWDGE, normalize round trips keep qSP latency low. xT/weight loads
    are single fused strided-AP descriptors, and xT is split in k-halves so
    the projection k-loop starts on the first half (subtile deps).
  - V^T is built by a regular fp16 matmul against a 128x128 identity (one
    N=128 matmul flips both heads of a 128-token tile) instead of two
    transpose-mode ops per tile: transpose-mode pays a ~275ns PSUM access
    latency and doesn't engage the fast clock.
  - ~2.4us of warmup matmuls at the top of the stream flip the PE HAM clock
    gate to 8/8 while the first DMAs are still in flight.
  - The last i-block of the last batch normalizes fully on-chip (denominator
    row -> [tok,1] via K=1 fp32 matmuls, reciprocal, per-head K=64 y matmuls
    scaled by a per-partition ACT/DVE multiply) so the kernel tail doesn't
    sit through the DRAM reciprocal-broadcast round trips.
  - PSUM evacuation copies rebalanced: ACT keeps exp + q/k/v evacuations,
    DVE takes everything else; y chain slots pushed to +8..+12 so the PE
    FIFO never parks on the normalize DMAs.

v11: the RoPE rot matmul runs as ONE full 128-contraction matmul (P2's
off-diagonal blocks are zero) instead of two 64x64 tiles -- measured
row/col-tiled matmul pairs do NOT co-issue on this toolchain (only ~9% won
the LDWEIGHTS race), so the single matmul has the same stream length with
one less instruction and no tiling-mode switch (mode changes drain the PE).
Weight/xT loads are half-split so projections start on half-landed tiles,
and the phase-A pump finishes ~3 steps early so the next batch's first
S^T matmul never waits on the last rope_b.

On-core layout (all "T" tensors feature-major: partitions=feature, free=tok):
  Q^T/K^T (128 x 2048/batch): rows = [h0 d-evens(32), h0 d-odds(32), h1 ...]
  RoPE: dst = qr*cos - P2@(qr*sin2); P2 is block-diagonal per head.
  S^T tiles (tj x ti) = K^T.T @ Q^T per head (64-contraction row pair).
    A = exp(0.125*S^T) in fp16 (max logit ~3.6 -> exp <= ~40, far from fp16
    overflow); diagonal straddle tiles are masked AFTER exp by a 0/1 fp16
    triangle multiply (2x DVE mode).
  V (tok-major, per 128-token tile): [d 0..63 | ones | zeros*63] per head;
    the ones column rides the AV matmul to produce softmax denominators and
    the zeros pad the lhsT to 128 (FWL). zeros/ones are written once.
  O~^T (65 x ti) accumulated = [V|1].T @ A over tj chunks; row 64 = softmax
    denominators. Normalize via DRAM-staged reciprocal broadcast + DVE mult
    (except the final block, which normalizes on-chip as described above).
  y partial (128t x 1024) = O^T-chunk.T @ Wo-slice, fp16, psum->sbuf->DRAM.
"""

import numpy as np

B, T, C = 4, 2048, 1024
H, D = 16, 64
N_CORES = 8
BT = B * T
SCALE = 0.125  # D**-0.5

TRACE = False            # set True (e.g. from test.py) to capture an NTFF trace
LAST_RESULT = None       # BassKernelResults of the most recent run

_BUILT = None            # cached nc


# --------------------------------------------------------------------------
# workaround: this walrus build rejects >1 semaphore wait per instruction
def _split_sem_waits(nc, max_waits=1):
    import concourse.mybir as mybir

    n = 0
    for f in nc.m.functions:
        for bb in f.blocks:
            insts = bb.instructions
            idx = 0
            while idx < len(insts):
                i = insts[idx]
                si = getattr(i, "sync_info", None)
                if si is not None and si.on_wait and len(si.on_wait) > max_waits:
                    waits = list(si.on_wait)
                    extra, keep = waits[:-max_waits], waits[-max_waits:]
                    si.on_wait = keep
                    pos = idx
                    for j in range(0, len(extra), max_waits):
                        n += 1
                        nd = mybir.InstNoOp(name=f"I-waitsplit-{n}", ins=[], outs=[])
                        nd.engine = i.engine
                        nd.sync_info = mybir.SyncInfo(
                            on_wait=extra[j : j + max_waits], on_update=[]
                        )
                        insts.insert(pos, nd)
                        pos += 1
                    idx = pos
                idx += 1


def _install_ntff_hook():
    """The image's antenv lacks axon_hooks; synthesize it so trace=True works."""
    import sys
    import types

    if "antenv.axon_hooks" in sys.modules:
        return
    import antenv

    state = {"hook": None}
    mod = types.ModuleType("antenv.axon_hooks")
    mod.get_axon_ntff_profile_hook = lambda: state["hook"]
    mod.set_axon_ntff_profile_hook = lambda h: state.__setitem__("hook", h)
    sys.modules["antenv.axon_hooks"] = mod
    antenv.axon_hooks = mod
    try:
        import contextlib

        from trn_agent_boot.trn_boot import _ntff_profile_via_ctypes

        inner = _ntff_profile_via_ctypes("/opt/axon/libaxon_pjrt.so")

        # axon_start_nrt_profile needs the PJRT client fully initialized;
        # retry with a forced execute between attempts, degrade to
        # no-profile rather than crash the whole run
        @contextlib.contextmanager
        def robust_hook(output_dir, device_ids):
            import jax.numpy as jnp

            ctx = None
            for attempt in range(3):
                try:
                    jnp.zeros(8).block_until_ready()
                    c = inner(output_dir, device_ids)
                    c.__enter__()
                    ctx = c
                    break
                except Exception as e:  # profile start failed; retry
                    print(f"NTFF start attempt {attempt} failed: {e}")
                    import time as _t

                    _t.sleep(1.0)
            if ctx is None:
                print("NTFF profile unavailable; running without trace")
            try:
                yield
            finally:
                if ctx is not None:
                    ctx.__exit__(None, None, None)

        state["hook"] = robust_hook if inner is not None else None
    except Exception:
        state["hook"] = None


# --------------------------------------------------------------------------
def _build():
    import concourse.bass as bass
    import concourse.mybir as mybir
    from concourse.tile import TileContext

    F = mybir.dt.float32
    MD = mybir.dt.float16  # matmul operand dtype
    MULT = mybir.AluOpType.mult
    SUB = mybir.AluOpType.subtract
    EXP = mybir.ActivationFunctionType.Exp

    nc = bass.Bass()

    xT = nc.dram_tensor("xT", (C, BT), MD, kind="ExternalInput")
    wq = nc.dram_tensor("wq", (C, 128), MD, kind="ExternalInput")
    wk = nc.dram_tensor("wk", (C, 128), MD, kind="ExternalInput")
    wv = nc.dram_tensor("wv", (C, 128), MD, kind="ExternalInput")
    wo = nc.dram_tensor("wo", (128, C), MD, kind="ExternalInput")
    cosd = nc.dram_tensor("cos", (128, T), MD, kind="ExternalInput")
    sind = nc.dram_tensor("sin2", (128, T), MD, kind="ExternalInput")
    p2d = nc.dram_tensor("p2", (128, 128), MD, kind="ExternalInput")
    trid = nc.dram_tensor("tri2x", (128, 256), MD, kind="ExternalInput")
    idd = nc.dram_tensor("id128", (128, 128), MD, kind="ExternalInput")
    y = nc.dram_tensor("y", (BT, C), MD, kind="ExternalOutput")
    scr = nc.dram_tensor("scr", (B * 8, 512), F, kind="Internal")

    with TileContext(nc) as tc:
        with (
            tc.tile_pool(name="const", bufs=1) as cst,
            tc.tile_pool(name="xt", bufs=4) as xtp,
            tc.tile_pool(name="qt", bufs=2) as qp,
            tc.tile_pool(name="kt", bufs=2) as kp,
            tc.tile_pool(name="ot", bufs=2) as op_,
            tc.tile_pool(name="tmp", bufs=4) as tmp,
            tc.tile_pool(name="at", bufs=6) as ap_,
            tc.tile_pool(name="bc", bufs=4) as bcp,
            tc.tile_pool(name="avs", bufs=4) as avsp,
            tc.tile_pool(name="rr", bufs=4) as rp,
            tc.tile_pool(name="ys", bufs=4) as ysp,
            tc.tile_pool(name="sps", bufs=2, space="PSUM") as sps,
            tc.tile_pool(name="stp", bufs=2, space="PSUM") as stp,
            tc.tile_pool(name="avp", bufs=2, space="PSUM") as avp,
        ):
            # ---- PE warmup: flip the HAM clock gate while DMAs land --------
            wup = cst.tile([128, 128], MD)
            nc.vector.memset(wup[:, :], 0.0)
            wps = sps.tile([128, 512], F, tag="s", name="wps")
            for _ in range(36):
                nc.tensor.matmul(
                    wps[:, 0:128], lhsT=wup[:, :], rhs=wup[:, :],
                    start=True, stop=True, skip_group_check=True,
                )

            # ---- constants (fused single-descriptor loads) -----------------
            # q/k/v weights + rope tables on the ACT HWDGE ring; the rest on
            # the SP ring so the prologue loads run on two queues in parallel.
            # ordered by first use: ACT ring carries wq/sin/cos/wk/wv (the
            # phase-A critical chain), SP ring carries p2/tri/wo in parallel.
            wq_t = cst.tile([128, 8, 128], MD)
            wk_t = cst.tile([128, 8, 128], MD)
            wv_t = cst.tile([128, 8, 128], MD)
            # half-split loads: the projection's k-loop can start on the
            # first four chunks while the rest land (subtile deps)
            for wt, wd in ((wq_t, wq), (wk_t, wk), (wv_t, wv)):
                ws = wd[:, :].rearrange("(k p) c -> p k c", p=128)
                nc.scalar.dma_start(out=wt[:, 0:4, :], in_=ws[:, 0:4, :])
                nc.scalar.dma_start(out=wt[:, 4:8, :], in_=ws[:, 4:8, :])
            p2_t = cst.tile([128, 128], MD)
            nc.sync.dma_start(out=p2_t, in_=p2d[:, :])
            sin_t = cst.tile([128, T], MD)
            nc.sync.dma_start(out=sin_t, in_=sind[:, :])
            cos_t = cst.tile([128, T], MD)
            nc.sync.dma_start(out=cos_t, in_=cosd[:, :])
            id_t = cst.tile([128, 128], MD)
            nc.sync.dma_start(out=id_t, in_=idd[:, :])
            onesf = cst.tile([128, 1], F)
            nc.vector.memset(onesf[:, :], 1.0)
            tri_t = cst.tile([128, 256], MD)  # [tri01 | tri01] for head pairs
            nc.sync.dma_start(out=tri_t, in_=trid[:, :])
            wo_t = cst.tile([128, C], MD)
            nc.sync.dma_start(out=wo_t, in_=wo[:, :])

            # persistent double-buffered token-major V storage: per 128-token
            # tile 256 cols [d 0..63 | ones | zeros*63 | d2 | ones | zeros*63];
            # the ones column rides the AV matmul to produce softmax
            # denominators and the zeros pad the lhsT to 128 (FWL).
            VBUF = []
            for vi in range(2):
                Vb = cst.tile([128, 16, 256], MD, name=f"Vb{vi}", tag=f"vb{vi}")
                nc.vector.memset(Vb[:, :, :], 0.0)
                nc.vector.memset(Vb[:, :, 64:256:128], 1.0)
                VBUF.append(Vb)

            QK = {}  # b -> (Qb, Kb)

            # ---- slotted deferred-work chain --------------------------------
            gstep = [0]
            chain_q = []  # (due_step, fn)

            def sched(delay, fn):
                chain_q.append((gstep[0] + delay, fn))

            def drain_chain():
                i = 0
                while i < len(chain_q):
                    due, fn = chain_q[i]
                    if due <= gstep[0]:
                        chain_q.pop(i)
                        fn()
                    else:
                        i += 1

            # ---- phase A: projections + rope + V, as pumpable units --------
            def make_a_units(b):
                Qb = qp.tile([128, T], MD, name="Qb")
                Kb = kp.tile([128, T], MD, name="Kb")
                QK[b] = (Qb, Kb)
                Vb = VBUF[b % 2]
                xts = {}
                holders = {}

                def xt_load(nb):
                    def f():
                        xt = xtp.tile([128, 8, 512], MD, name="xt")
                        xts[nb] = xt
                        g0 = b * T + nb * 512
                        src = xT[:, g0 : g0 + 512].rearrange(
                            "(k p) t -> p k t", p=128)
                        # batch 0 rides the otherwise-empty gpsimd ring (the
                        # HWDGE rings are busy with constants at the
                        # prologue); later batches trigger from the idle SP
                        # engine so the gpsimd ring carries only y stores.
                        # Split loads: subtile deps let the projection k-loop
                        # start on the first chunk; batch 0's first tile is
                        # quarter-split so the very first matmul starts early.
                        eng = nc.gpsimd if b == 0 else nc.sync
                        if b == 0 and nb == 0:
                            for kq in range(4):
                                eng.dma_start(out=xt[:, 2 * kq : 2 * kq + 2, :],
                                              in_=src[:, 2 * kq : 2 * kq + 2, :])
                        else:
                            eng.dma_start(out=xt[:, 0:4, :], in_=src[:, 0:4, :])
                            eng.dma_start(out=xt[:, 4:8, :], in_=src[:, 4:8, :])
                    return f

                def proj(W_t, key, nb, half):
                    def f():
                        if half == 0:
                            holders[key] = sps.tile([128, 512], F, tag="s", name="ps")
                        ps = holders[key]
                        xt = xts[nb]
                        for k in range(4 * half, 4 * half + 4):
                            nc.tensor.matmul(
                                ps[:, :], lhsT=W_t[:, k, :], rhs=xt[:, k, :],
                                start=(k == 0), stop=(k == 7),
                                skip_group_check=True,
                            )
                    return f

                def rope_a(key, nb):
                    # evacuate + elementwise half of rope; frees the ps bank.
                    # DVE, not ACT: the ACT FIFO carries only exp so the
                    # st->exp->av loop never parks behind a 720ns copy.
                    def f():
                        ps = holders[key]
                        qr = tmp.tile([128, 512], MD, name="qr")
                        nc.vector.tensor_copy(qr[:, :], ps[:, :])
                        qs = tmp.tile([128, 512], MD, name="qs")
                        nc.vector.tensor_tensor(
                            qs[:, :], qr[:, :], sin_t[:, nb * 512 : (nb + 1) * 512],
                            MULT)
                        holders[key] = (qr, qs)
                    return f

                def rope_b(key, nb, dstb):
                    def f():
                        qr, qs = holders.pop(key)
                        cols = slice(nb * 512, (nb + 1) * 512)
                        nc.vector.tensor_tensor(dstb[:, cols], qr[:, :],
                                                cos_t[:, cols], MULT)
                        rot = sps.tile([128, 512], F, tag="s", name="rot")
                        # P2 is block-diagonal with zero off-diagonal blocks,
                        # so one full 128-contraction matmul computes both
                        # heads -- same stream length, no 64x64 tiling-mode
                        # switch (mode changes drain the PE array)
                        nc.tensor.matmul(
                            rot[:, :], lhsT=p2_t[:, :], rhs=qs[:, :],
                            start=True, stop=True,
                        )
                        nc.vector.tensor_tensor(dstb[:, cols], dstb[:, cols],
                                                rot[:, :], SUB)
                    return f

                # feature-major V projection (N=512 matmuls), fp16 PSUM
                # evacuation, then token-major transpose as a regular fp16
                # matmul against the identity: one N=128 matmul flips both
                # heads of a 128-token tile at once (vs 2 transpose-mode ops).
                def vst_unit(nb):
                    def f():
                        ps = holders.pop("v")
                        vst = tmp.tile([128, 512], MD, name="vst", tag="vst")
                        holders["vst"] = vst
                        nc.vector.tensor_copy(vst[:, :], ps[:, :])
                    return f

                def vtrans(nb, tl):
                    def f():
                        vst = holders["vst"]
                        tt = nb * 4 + tl
                        tcs = slice(tl * 128, (tl + 1) * 128)
                        tp = sps.tile([128, 128], F, tag="s", name="tp")
                        nc.tensor.matmul(
                            tp[:, :], lhsT=vst[:, tcs], rhs=id_t[:, :],
                            start=True, stop=True,
                        )
                        nc.vector.tensor_copy(
                            Vb[:, tt, :].rearrange(
                                "p (h d) -> p h d", h=2)[:, :, 0:64],
                            tp[:, :].rearrange(
                                "p (h d) -> p h d", h=2)[:, :, 0:64],
                        )
                    return f

                xt_load(0)()  # eager: max DMA lead for the first chunk
                units = []
                for nb in range(4):
                    units.append(proj(wq_t, "q", nb, 0))
                    units.append(proj(wq_t, "q", nb, 1))
                    units.append(rope_a("q", nb))
                    if nb + 1 < 4:
                        units.append(xt_load(nb + 1))
                    units.append(rope_b("q", nb, Qb))
                    units.append(proj(wk_t, "k", nb, 0))
                    units.append(proj(wk_t, "k", nb, 1))
                    units.append(rope_a("k", nb))
                    units.append(rope_b("k", nb, Kb))
                    units.append(proj(wv_t, "v", nb, 0))
                    units.append(proj(wv_t, "v", nb, 1))
                    units.append(vst_unit(nb))
                    for tl in range(4):
                        units.append(vtrans(nb, tl))
                return units

            # ---- y projection for one 128-token tile, split in two slots ---
            def y_half(b, Ob, i, tl, nh, ysb_h):
                def f():
                    tt = 4 * i + tl
                    lhs = Ob[:, tt * 128 : (tt + 1) * 128]
                    if nh == 0:
                        ysb_h["t"] = ysp.tile([128, 1024], MD, name="ysb")
                    ysb = ysb_h["t"]
                    yps = sps.tile([128, 512], F, tag="s", name="yps")
                    nc.tensor.matmul(
                        yps[:, :], lhsT=lhs,
                        rhs=wo_t[:, nh * 512 : (nh + 1) * 512],
                        start=True, stop=True,
                    )
                    nc.vector.tensor_copy(
                        ysb[:, nh * 512 : (nh + 1) * 512], yps[:, :])
                    if nh == 1:
                        r0 = b * T + tt * 128
                        # last batch stores on the (by then empty) SP ring so
                        # the end-of-kernel barrier isn't waiting on the
                        # gpsimd software queue to drain
                        eng = nc.sync if b == B - 1 else nc.gpsimd
                        eng.dma_start(out=y[r0 : r0 + 128, :], in_=ysb[:, :])
                return f

            # ---- phase D: attention for batch b, pumping `units` ------------
            def phase_d(b, units):
                Qb, Kb = QK[b]
                Vb = VBUF[b % 2]
                Ob = op_.tile([128, T], MD, name="Ob")
                steps_left = [40]

                def pump():
                    gstep[0] += 1
                    steps_left[0] -= 1
                    drain_chain()
                    # finish the pumped units ~3 steps before the batch ends
                    # so the next batch's first st never waits on the last
                    # rope_b
                    eff = max(steps_left[0] - 3, 1) if steps_left[0] > 0 else 0
                    if units and eff > 0:
                        n = -(-len(units) // eff)
                        for _ in range(min(n, len(units))):
                            units.pop(0)()
                    elif units:
                        while units:
                            units.pop(0)()

                for i in range(4):
                    av = [avp.tile([128, 512], F, tag="av", name="av")
                          for _ in (0, 1)]
                    nch = 4 * i + 4
                    sts = {}
                    As = {}

                    def emit_st(j):
                        delta = j * 128 - i * 512
                        nl = 512 - max(0, delta)
                        off = 512 - nl
                        st = stp.tile([128, 2, 512], F, name="st")
                        for h in (0, 1):
                            hs = slice(64 * h, 64 * h + 64)
                            nc.tensor.matmul(
                                st[:, h, 0:nl],
                                lhsT=Kb[hs, j * 128 : (j + 1) * 128],
                                rhs=Qb[hs, i * 512 + off : (i + 1) * 512],
                                start=True, stop=True,
                            )
                        sts[j] = (st, off, nl)

                    def emit_exp(j):
                        st, off, nl = sts.pop(j)
                        A = ap_.tile([128, 2, 512], MD, name="A")
                        nc.scalar.activation(
                            A[:, :, 0:nl], st[:, :, 0:nl], EXP, scale=SCALE)
                        if j * 128 >= i * 512:  # diagonal straddle: zero the
                            # above-diagonal triangle (first 128 cols) post-exp
                            nc.vector.tensor_tensor(
                                A[:, :, 0:128], A[:, :, 0:128],
                                tri_t[:, :].rearrange("p (a c) -> p a c", a=2),
                                MULT)
                        As[j] = (A, off, nl)

                    def emit_av(j):
                        A, off, nl = As.pop(j)
                        for h in (0, 1):
                            nc.tensor.matmul(
                                av[h][0:128, off:512],
                                lhsT=Vb[:, j, 128 * h : 128 * h + 128],
                                rhs=A[:, h, 0:nl],
                                start=(j == 0), stop=(j == nch - 1),
                                skip_group_check=True,
                            )

                    emit_st(0)
                    if nch > 1:
                        emit_st(1)
                    emit_exp(0)
                    for j in range(nch):
                        if j + 1 < nch:
                            emit_exp(j + 1)
                        if j + 2 < nch:
                            emit_st(j + 2)
                        pump()
                        emit_av(j)

                    # normalize chain for this i-block, slotted so no engine
                    # FIFO parks behind the DRAM reciprocal round trip
                    def norm_a(i=i, av=av):
                        for h in (0, 1):
                            avs = avsp.tile([65, 512], F, name="avs")
                            nc.vector.tensor_copy(avs[:, :], av[h][0:65, :])
                            srt = rp.tile([128, 4], F, name="srt")
                            nc.sync.dma_start(out=srt[:, :], in_=avs[64:65, :])
                            norm_state[(b, i, h)] = [avs, srt, None]

                    def norm_b(i=i):
                        for h in (0, 1):
                            row = b * 8 + i * 2 + h
                            avs, srt, _ = norm_state[(b, i, h)]
                            rt = rp.tile([128, 4], F, name="rt")
                            nc.vector.reciprocal(rt[:, :], srt[:, :])
                            nc.sync.dma_start(
                                out=scr[row : row + 1, :].rearrange(
                                    "r (p c) -> (r p) c", c=4),
                                in_=rt[:, :],
                            )
                            bct = bcp.tile([64, 512], F, name="bct")
                            src = scr[row : row + 1, :]
                            bap = bass.AP(
                                tensor=src.tensor, offset=src.offset,
                                ap=[[0, 64]] + [list(p) for p in src.ap[1:]],
                            )
                            nc.sync.dma_start(out=bct[:, :], in_=bap)
                            norm_state[(b, i, h)][2] = bct

                    def norm_c(i=i, Ob=Ob):
                        for h in (0, 1):
                            avs, srt, bct = norm_state.pop((b, i, h))
                            nc.vector.tensor_tensor(
                                Ob[64 * h : 64 * h + 64,
                                   i * 512 : (i + 1) * 512],
                                avs[0:64, :], bct[:, :], MULT,
                            )

                    if b == B - 1 and i == 3:
                        # final i-block: normalize + project entirely on-chip
                        # (no DRAM reciprocal round trips in the kernel tail).
                        # Denominator row -> [tok,1] layout via tiny fp32
                        # transpose matmuls, reciprocal, then per-head y
                        # projection scaled by the per-partition reciprocal.
                        ADD = mybir.AluOpType.add
                        avs_h = []
                        for h in (0, 1):
                            avs = avsp.tile([65, 512], F, name="avs")
                            nc.vector.tensor_copy(avs[:, :], av[h][0:65, :])
                            nc.vector.tensor_copy(
                                Ob[64 * h : 64 * h + 64,
                                   i * 512 : (i + 1) * 512],
                                avs[0:64, :])  # unnormalized O~ in fp16
                            avs_h.append(avs)
                        tpd = sps.tile([128, 8], F, tag="s", name="tpd")
                        for h in (0, 1):
                            for tl in range(4):
                                c = h * 4 + tl
                                nc.tensor.matmul(
                                    tpd[:, c : c + 1],
                                    lhsT=avs_h[h][64:65,
                                                  tl * 128 : (tl + 1) * 128],
                                    rhs=onesf[64:65, :],
                                    start=(c == 0), stop=(c == 7),
                                    skip_group_check=True,
                                )
                        rtt = rp.tile([128, 8], F, name="rtt", tag="rtt")
                        nc.vector.reciprocal(rtt[:, :], tpd[:, :])
                        for tl in range(4):
                            tt = 4 * i + tl
                            ysb = ysp.tile([128, 1024], MD, name="ysb")
                            for nh in (0, 1):
                                yph = []
                                for h in (0, 1):
                                    yps = sps.tile([128, 512], F, tag="s",
                                                   name="yps")
                                    nc.tensor.matmul(
                                        yps[:, :],
                                        lhsT=Ob[64 * h : 64 * h + 64,
                                                tt * 128 : (tt + 1) * 128],
                                        rhs=wo_t[64 * h : 64 * h + 64,
                                                 nh * 512 : (nh + 1) * 512],
                                        start=True, stop=True,
                                        tile_position=(64 * h, 0),
                                    )
                                    yph.append(yps)
                                t0f = tmp.tile([128, 512], F, name="t0f",
                                               tag="tf")
                                t1f = tmp.tile([128, 512], F, name="t1f",
                                               tag="tf")
                                nc.scalar.mul(t0f[:, :], yph[0][:, :],
                                              rtt[:, tl : tl + 1])
                                nc.vector.tensor_scalar(
                                    t1f[:, :], yph[1][:, :],
                                    rtt[:, 4 + tl : 5 + tl], None, MULT)
                                nc.vector.tensor_tensor(
                                    ysb[:, nh * 512 : (nh + 1) * 512],
                                    t0f[:, :], t1f[:, :], ADD)
                            r0 = b * T + tt * 128
                            nc.sync.dma_start(out=y[r0 : r0 + 128, :],
                                              in_=ysb[:, :])
                    else:
                        # norm_a runs NOW (not slotted): its PSUM-evacuation
                        # copies land in the DVE FIFO ahead of all next-block
                        # work, so the next block's first AV matmul (which
                        # reuses these av banks) stalls on nothing
                        norm_a()
                        sched(3, norm_b)
                        sched(6, norm_c)
                        ysb_hs = [{} for _ in range(4)]
                        for tl in range(4):
                            sched(8 + tl, y_half(b, Ob, i, tl, 0, ysb_hs[tl]))
                            sched(9 + tl, y_half(b, Ob, i, tl, 1, ysb_hs[tl]))

            norm_state = {}

            # ---- top-level schedule ----------------------------------------
            for u in make_a_units(0):
                u()
            for b in range(B):
                units = make_a_units(b + 1) if b + 1 < B else []
                phase_d(b, units)
            while chain_q:
                gstep[0] += 1
                drain_chain()

    _split_sem_waits(nc)
    return nc


# --------------------------------------------------------------------------
def _host_inputs(x, Wq, Wk, Wv):
    """Per-core input dicts (all shared arrays built once)."""
    BF = np.float16
    xT = np.ascontiguousarray(
        np.asarray(x, dtype=np.float32).reshape(BT, C).T).astype(BF)

    # NeoX d-permutation within each head: evens then odds
    dperm = np.concatenate([np.arange(0, D, 2), np.arange(1, D, 2)])

    inv_freq = (1.0 / (10000.0 ** (np.arange(0, D, 2) / D))).astype(np.float64)
    pos = np.arange(T, dtype=np.float64)
    ang = pos[None, :] * inv_freq[:, None]  # (32, T)
    cos32 = np.cos(ang).astype(np.float32)
    sin32 = np.sin(ang).astype(np.float32)
    cos_t = np.tile(np.vstack([cos32, cos32]), (2, 1))  # (128, T)
    sin_t = np.tile(np.vstack([-sin32, sin32]), (2, 1))  # (128, T), sign folded

    p2 = np.zeros((128, 128), dtype=np.float32)
    for hb in (0, 64):
        for i2 in range(32):
            p2[hb + i2, hb + 32 + i2] = 1.0
            p2[hb + 32 + i2, hb + i2] = 1.0

    tri = np.where(
        np.arange(128)[None, :] >= np.arange(128)[:, None], 1.0, 0.0
    ).astype(np.float32)
    tri2x = np.concatenate([tri, tri], axis=1)  # (128, 256)

    Wq = np.asarray(Wq, dtype=np.float32)
    Wk = np.asarray(Wk, dtype=np.float32)
    Wv = np.asarray(Wv, dtype=np.float32)

    in_maps = []
    for c in range(N_CORES):
        sl = slice(128 * c, 128 * (c + 1))
        wq_c = Wq[:, sl].reshape(C, 2, D)[:, :, dperm].reshape(C, 128)
        wk_c = Wk[:, sl].reshape(C, 2, D)[:, :, dperm].reshape(C, 128)
        in_maps.append({
            "xT": xT,
            "wq": np.ascontiguousarray(wq_c).astype(BF),
            "wk": np.ascontiguousarray(wk_c).astype(BF),
            "wv": np.ascontiguousarray(Wv[:, sl]).astype(BF),
            "wo": None,  # set below
            "cos": cos_t.astype(BF),
            "sin2": sin_t.astype(BF),
            "p2": p2.astype(BF),
            "tri2x": tri2x.astype(BF),
            "id128": np.eye(128, dtype=np.float32).astype(BF),
        })
    return in_maps


def kernel(x, Wq, Wk, Wv, Wo, bo):
    global _BUILT, LAST_RESULT
    from concourse.bass_utils import run_bass_kernel_spmd

    if TRACE:
        _install_ntff_hook()

    if _BUILT is None:
        _BUILT = _build()
    nc = _BUILT

    in_maps = _host_inputs(x, Wq, Wk, Wv)
    Wo = np.asarray(Wo, dtype=np.float32)
    for c in range(N_CORES):
        in_maps[c]["wo"] = np.ascontiguousarray(
            Wo[128 * c : 128 * (c + 1), :]).astype(np.float16)

    last_err = None
    for attempt in range(3):
        try:
            res = run_bass_kernel_spmd(
                nc, in_maps, core_ids=list(range(N_CORES)), trace=TRACE
            )
            break
        except Exception as e:  # transient NRT device errors: retry
            last_err = e
            import time as _time

            _time.sleep(2.0)
    else:
        raise last_err
    LAST_RESULT = res

    acc = res.results[0]["y"].astype(np.float32)
    for c in range(1, N_CORES):
        acc = acc + res.results[c]["y"].astype(np.float32)
    out = acc + np.asarray(bo, dtype=np.float32)[None, :]
    return out.reshape(B, T, C)
